# revision 24
# baseline (speedup 1.0000x reference)
"""NT-Xent loss kernel for Trainium2, 8 NeuronCores.

Problem: B=4096 per view, D=128, temperature=0.1.
reps = concat([zjs, zis]) -> [2B, D] = [8192, 128]; normalize rows;
sim = normed @ normed.T; loss = mean_i(-pos_i/T + logsumexp_{j!=i}(sim_ij/T)).

Fully static SPMD, no collectives; sim symmetry halves the exp work.
8192 rows = 64 tiles of 128.  Row tile T computes the column band
[T, T+32]; the diagonal tile is covered by row sums alone; tiles
T+1..T+31 contribute row AND column sums; tile T+32 both at weight 0.5
(pairs at tile distance 32 are computed from both sides).  Per core: 8
row tiles; the rotated input keeps the band contiguous.

The device runs exactly two stages, sized so the ACT exp stream (the
hard floor: 33.8k psum columns/core at 0.83ns each) never stalls:
  PE   fp8e4m3 sim matmuls -> PSUM   (~3.3us per row tile)
  ACT  exp(10x-4.6) PSUM -> SBUF fp8 (~4.3us per row tile)
  DMA  every E tile streams to DRAM on otherwise-idle DMA queues
All reductions happen on the host in f64: row sums (excluding the
dumped self element exactly — no exp replication needed), column sums
for tiles T+1.., 0.5 weight on the distance-32 tail, minus pos/diag
dots from the same fp8 inputs the matmul contracted.  That host work is
~34M adds, 0.2% of the device FLOPs.  fp8 inputs (validated 9e-5 rel
err) halve the input DMA that gates the pipeline start; the fp8 E dump
(shift 4.6 keeps exp in fp8 range) halves the outbound stream.
"""

import numpy as np

B = 4096
D = 128
TWO_B = 2 * B
P = 128
NCORES = 8
ROWS_PER_CORE = TWO_B // NCORES  # 1024
MI = 8                    # row tiles per core (128 rows each)
NTILES_IN = 44            # band cols reach local tile 40; pad to 44
NSLICES = NTILES_IN // 4
STRIPS = ((0, 1536), (1536, 1536), (3072, 1152))
INV_T = 10.0              # 1 / temperature
SHIFT = 10.0              # logsumexp shift in the combine frame
SHIFT8 = 4.6              # shift for the fp8 E dump (max e^{10*0.6-4.6} ~ 4)

_CACHE = {}


def build_nc():
    import concourse.bacc as bacc
    import concourse.mybir as mybir
    import concourse.tile as tile

    f32 = mybir.dt.float32
    bf16 = mybir.dt.bfloat16
    fp8 = mybir.dt.float8e4
    AF = mybir.ActivationFunctionType

    # Pin the act-table chooser to the one set that holds Exp so no
    # mid-kernel ACT_TABLE_LOADs are emitted.
    from concourse import hw_specs

    _orig_tables = hw_specs.get_activation_tables

    def _patched_tables(arch):
        t = {k: set(v) for k, v in _orig_tables(arch).items()}
        for name, s in t.items():
            if name != "natural_log_exp_and_others":
                s.discard(AF.Exp)
                s.discard(AF.Ln)
        return t

    bacc.get_activation_tables = _patched_tables

    nc = bacc.Bacc(
        "TRN2",
        target_bir_lowering=False,
        debug=False,
        num_devices=NCORES,
    )
    # hit[d, 128t+p] = fp8e4m3(normed_rot[128t+p, d])  (transposed layout)
    hit_h = nc.declare_dram_parameter("hit", [P, NTILES_IN * P], fp8,
                                      isOutput=False)
    # E dump: 24 strips of [128, 1536] fp8, order (B, A, C) per row tile
    ed_h = nc.declare_dram_parameter("edump", [P, 24 * 1536], fp8,
                                     isOutput=True)

    with tile.TileContext(nc) as tc:
        with (
            tc.tile_pool(name="persist", bufs=1) as persist,
            tc.tile_pool(name="psum", bufs=2, space="PSUM") as psum,
        ):
            HIT = persist.tile([P, NTILES_IN * P], fp8)
            ZER = persist.tile([P, P], bf16)
            ZW = persist.tile([P, 512], bf16)
            E8TILES = [
                persist.tile([P, 1536], fp8, name=f"E8{j}")
                for j in range(24)
            ]
            bias_shift8 = persist.tile([P, 1], f32)
            nc.vector.memset(ZER, 0.0)
            nc.vector.memset(ZW, 0.0)
            nc.vector.memset(bias_shift8, -SHIFT8)

            # ---------------- loads + PE warm-up ----------------------------
            # B(0) runs first and needs slices 3-5; A(0) needs 0-2.  The
            # scalar queue is left free so the ACT table loads run early.
            # A tiny first DMA on each queue absorbs the DGE init latency
            # before the slices that gate the pipeline start.
            dmaq = [nc.gpsimd, nc.sync]
            for q in dmaq:
                q.dma_start(out=HIT[:, 5628:5632], in_=hit_h[:, 5628:5632])
            order = [3, 4, 5, 0, 1, 2] + list(range(6, NSLICES))
            for qi, s in enumerate(order):
                x, y = 4 * s * P, (4 * s + 4) * P
                dmaq[qi % 2].dma_start(out=HIT[:, x:y], in_=hit_h[:, x:y])
            # warm the PE during the load phase on zero inputs (no DMA dep)
            WARM = psum.tile([P, 1536], f32, tag="pg")
            for _ in range(3):
                nc.tensor.matmul(WARM[:, 0:512], ZER, ZW,
                                 start=True, stop=True)

            # ---------------- per row tile: sims + exp + dump ---------------
            for t in range(MI):
                base = P * t
                # B, A, C: the first strip overall only needs DMA slices 3-5
                for pos, si in enumerate((1, 0, 2)):
                    off, w = STRIPS[si]
                    pg = psum.tile([P, 1536], f32, tag="pg")
                    for k in range(0, w, 512):
                        kw = min(512, w - k)
                        nc.tensor.matmul(
                            pg[:, k : k + kw],
                            HIT[:, base : base + P],
                            HIT[:, base + off + k : base + off + k + kw],
                            start=True, stop=True,
                        )
                    j = 3 * t + pos
                    E8 = E8TILES[j]
                    nc.scalar.activation(
                        out=E8[:, :w], in_=pg[:, :w], func=AF.Exp,
                        scale=INV_T, bias=bias_shift8,
                    )
                    if j == 23:
                        # split the final dump across both queues so its
                        # transfer drains ~2x faster after the last exp
                        h = w // 2
                        dmaq[0].dma_start(
                            out=ed_h[:, j * 1536 : j * 1536 + h],
                            in_=E8[:, :h],
                        )
                        dmaq[1].dma_start(
                            out=ed_h[:, j * 1536 + h : j * 1536 + w],
                            in_=E8[:, h:w],
                        )
                    else:
                        dmaq[j % 2].dma_start(
                            out=ed_h[:, j * 1536 : j * 1536 + w],
                            in_=E8[:, :w],
                        )

    nc.compile()
    return nc


def get_nc():
    if "nc" not in _CACHE:
        _CACHE["nc"] = build_nc()
    return _CACHE["nc"]


def _prep(zis: np.ndarray, zjs: np.ndarray):
    import ml_dtypes

    # representations in reference order: [zjs; zis], normalized rows
    # (f32 norms with the torch CosineSimilarity 1e-8 clamp), quantized to
    # fp8e4m3 — the exact values the device matmul contracts over.
    reps = np.concatenate(
        [np.asarray(zjs, np.float32), np.asarray(zis, np.float32)], axis=0
    )
    normed = (
        reps / np.maximum(np.linalg.norm(reps, axis=1, keepdims=True), 1e-8)
    ).astype(ml_dtypes.float8_e4m3)
    return normed


def make_in_maps(zis: np.ndarray, zjs: np.ndarray):
    normed = _prep(zis, zjs)
    maps = []
    for c in range(NCORES):
        rot = np.roll(normed, -ROWS_PER_CORE * c, axis=0)[: NTILES_IN * P]
        maps.append({"hit": np.ascontiguousarray(rot.T)})
    return maps


def kernel(zis: np.ndarray, zjs: np.ndarray) -> np.ndarray:
    from concourse.bass_utils import run_bass_kernel_spmd

    nc = get_nc()
    normed = _prep(zis, zjs)
    maps = []
    for c in range(NCORES):
        rot = np.roll(normed, -ROWS_PER_CORE * c, axis=0)[: NTILES_IN * P]
        maps.append({"hit": np.ascontiguousarray(rot.T)})

    res = None
    for attempt in range(3):
        try:
            res = run_bass_kernel_spmd(nc, maps, core_ids=list(range(NCORES)))
            break
        except Exception:
            # transient device-unrecoverable states heal on re-execution
            if attempt == 2:
                raise
            import time as _time

            _time.sleep(5.0)

    # ---- host combine (f64) -------------------------------------------
    nf = normed.astype(np.float64)
    pos = np.sum(nf * np.roll(nf, -B, axis=0), axis=1)   # h_i . h_{(i+B)%2B}

    r = np.zeros(TWO_B, dtype=np.float64)
    s8 = np.exp(SHIFT8 - SHIFT)       # rescale the dump to the shift-10 frame

    p_idx = np.arange(P)
    t_idx = np.arange(MI)
    row_l = 128 * t_idx[None, :] + p_idx[:, None]              # [P, MI]
    cwC = np.ones(1152)
    cwC[1024:] = 0.5                                           # C tail weight

    for c, rr in enumerate(res.results):
        ed = rr["edump"].astype(np.float32).reshape(P, MI, 3, 1536)
        eb = ed[:, :, 0, :]                                    # strip B
        ea = ed[:, :, 1, :]                                    # strip A
        ec = ed[:, :, 2, :1152]                                # strip C
        # zero the self elements (col p of the diag tile in strip A) BEFORE
        # summing — exact self-exclusion, immune to the saturated diag exp
        ea[p_idx[:, None], t_idx[None, :], p_idx[:, None]] = 0.0

        g_row = (1024 * c + row_l) % TWO_B                     # [P, MI]
        rsum = (ea.astype(np.float64).sum(axis=2)
                + eb.astype(np.float64).sum(axis=2)
                + ec.astype(np.float64) @ cwC)
        np.add.at(r, g_row, s8 * rsum)

        # column sums: A cols 128.. (diag tile covered by row sums), B all,
        # C with 0.5 on the tail block
        caw = ea.astype(np.float64).sum(axis=0)                # [MI, 1536]
        cbw = eb.astype(np.float64).sum(axis=0)
        ccw = ec.astype(np.float64).sum(axis=0) * cwC
        for t in range(MI):
            ga = (1024 * c + 128 * t + 128 + np.arange(1408)) % TWO_B
            np.add.at(r, ga, s8 * caw[t, 128:])
            gb = (1024 * c + 128 * t + 1536 + np.arange(1536)) % TWO_B
            np.add.at(r, gb, s8 * cbw[t])
            gc = (1024 * c + 128 * t + 3072 + np.arange(1152)) % TWO_B
            np.add.at(r, gc, s8 * ccw[t])

    lse = np.log(r) + SHIFT
    loss = np.mean(-INV_T * pos + lse)
    return np.array(loss, dtype=np.float32)
